# revision 11
# baseline (speedup 1.0000x reference)
"""Multi-head attention Trainium2 kernel (8 NeuronCores), v1.

Sharding: core c handles batch b=c//4 and head group g=c%4 (4 of 16 heads,
as 2 head-pairs p=0,1).  Fully "transposed" formulation (no on-device
transposes):
  qT/kT [dq, s] via lhsT=W-pair, rhs=X^T;  v [s, dk] via lhsT=X^T-chunk,
  rhs=Wv;  scoresT [s_k, s_q] via lhsT=kT-chunk, rhs=qT (softmax axis =
  partition dim); exp fused on ScalarE (scale=1/8); rowsum via 64 ones
  columns packed next to v (free in PE column-passes); oT [dk, s_q] is the
  lhsT the output projection wants.

Differences vs v0 baseline (359 us):
  - Host pre-arranges ALL inputs in SBUF layout and bf16 where the PE needs
    bf16: no conversion DMAs, no scatter descriptors, half the input bytes.
    X tensors are s-block-major so each 512-s-block is one contiguous DMA.
  - AllToAll (8 ranks, 2x, one per head-pair) keeps the duplicate-quarters
    send, but the receiver now selects its batch's half with 2 cheap DVE ops
    (host-supplied 0/1 scalars) instead of a zero-padded Wo: output
    projection contraction halves to 8 chunks, wo DMA halves.
  - cc_out tensors are addr_space="Shared" (fast HBM-HBM collective path).
  - Emission order = scheduler priority tuned so the kernel starts the exp
    stream ~12us in and the PE never waits on collectives until the tail.
"""

import sys

if "/opt/trn_rl_repo" not in sys.path:
    sys.path.insert(0, "/opt/trn_rl_repo")

import numpy as np
import ml_dtypes

import concourse.bass as bass  # noqa: F401
import concourse.bacc as bacc
import concourse.bass_utils as bass_utils
import concourse.mybir as mybir
import concourse.tile as tile

B, S, DIN = 2, 2048, 1024
H, DK = 16, 64
NCORES = 8
HL = 4  # heads per core
SQ = S // 4  # output rows per core
DC = DIN // 128  # 8 din chunks
SKC = S // 128  # 16 s_k chunks
VW = 2 * DK  # 128: 64 v columns + 64 ones columns (rowsum via PE)

F32 = mybir.dt.float32
BF16 = mybir.dt.bfloat16
BF = ml_dtypes.bfloat16


def build(dbg=False):
    nc = bacc.Bacc("TRN2", target_bir_lowering=False, debug=False, num_devices=NCORES)

    # ---- DRAM tensors (all host-prearranged, see make_in_maps) ----
    xq = nc.dram_tensor("xq", [128, 4 * DC * 512], BF16, kind="ExternalInput")
    xk = nc.dram_tensor("xk", [128, 4 * DC * 512], BF16, kind="ExternalInput")
    xv = nc.dram_tensor("xv", [128, 4 * DC * 512], BF16, kind="ExternalInput")
    wq = nc.dram_tensor("wq", [128, DC * 256], BF16, kind="ExternalInput")
    wk = nc.dram_tensor("wk", [128, DC * 256], BF16, kind="ExternalInput")
    wv = nc.dram_tensor("wv", [128, DC * 256], BF16, kind="ExternalInput")
    wo = nc.dram_tensor("wo", [128, DC * DIN], BF16, kind="ExternalInput")
    bqp = nc.dram_tensor("bqp", [128, 2], F32, kind="ExternalInput")
    bkp = nc.dram_tensor("bkp", [128, 2], F32, kind="ExternalInput")
    bvr = nc.dram_tensor("bvr", [128, HL * DK], F32, kind="ExternalInput")
    bor = nc.dram_tensor("bor", [128, DIN], F32, kind="ExternalInput")
    sel = nc.dram_tensor("sel", [128, 2], F32, kind="ExternalInput")
    out = nc.dram_tensor("out", [SQ, DIN], F32, kind="ExternalOutput")

    cc_in = [
        nc.dram_tensor(f"cc_in{p}", [8 * 2 * DK, SQ], BF16, kind="Internal")
        for p in range(2)
    ]
    cc_out = [
        nc.dram_tensor(f"cc_out{p}", [8 * 2 * DK, SQ], BF16, kind="Internal")
        for p in range(2)
    ]
    if dbg:
        d_qt = nc.dram_tensor("d_qt", [128, S], BF16, kind="ExternalOutput")
        d_v = nc.dram_tensor("d_v", [128, SKC * HL * VW], BF16, kind="ExternalOutput")
        d_ols = nc.dram_tensor("d_ols", [128, DC * SQ], BF16, kind="ExternalOutput")

    with tile.TileContext(nc) as tc:
        with (
            tc.tile_pool(name="pers", bufs=1) as pers,
            tc.tile_pool(name="work", bufs=3) as work,
            tc.tile_pool(name="wrk2", bufs=2) as wrk2,
            tc.tile_pool(name="recv", bufs=1) as recv,
            tc.tile_pool(name="psmm", bufs=2, space="PSUM") as psmm,
            tc.tile_pool(name="psacc", bufs=2, space="PSUM") as psacc,
            tc.tile_pool(name="pspj", bufs=2, space="PSUM") as pspj,
        ):
            # ---- small per-partition constants (sync queue) ----
            bq_sb = pers.tile([128, 2], F32)
            bk_sb = pers.tile([128, 2], F32)
            bv_sb = pers.tile([128, HL, DK], F32)
            bo_sb = pers.tile([128, DIN], F32)
            sel_sb = pers.tile([128, 2], F32)
            # ---- weights first (vector HWDGE queue, starts immediately) ----
            wq_sb = pers.tile([128, DC, 256], BF16)
            wk_sb = pers.tile([128, DC, 256], BF16)
            wv_sb = pers.tile([128, DC, 256], BF16)
            nc.scalar.dma_start(wq_sb[:], wq.rearrange("p (c d) -> p c d", c=DC))
            nc.scalar.dma_start(wk_sb[:], wk.rearrange("p (c d) -> p c d", c=DC))
            nc.sync.dma_start(bq_sb[:], bqp[:])
            nc.sync.dma_start(bk_sb[:], bkp[:])
            nc.sync.dma_start(bv_sb[:], bvr.rearrange("p (h d) -> p h d", h=HL))
            nc.sync.dma_start(bo_sb[:], bor[:])
            nc.sync.dma_start(sel_sb[:], sel[:])

            # ---- X loads, s-block-major streaming ----
            xq_sb = pers.tile([128, 4, DC, 512], BF16, name="xq_sb")
            xk_sb = pers.tile([128, 4, DC, 512], BF16, name="xk_sb")
            xv_sb = pers.tile([128, 4, DC, 512], BF16, name="xv_sb")

            def load_x(xsb, xdram, sblk):
                nc.gpsimd.dma_start(
                    xsb[:, sblk, :, :],
                    xdram[:, 4096 * sblk : 4096 * (sblk + 1)].rearrange(
                        "p (c s) -> p c s", c=DC
                    ),
                )

            load_x(xq_sb, xq, 0)
            load_x(xk_sb, xk, 0)
            load_x(xq_sb, xq, 1)
            load_x(xk_sb, xk, 1)
            nc.scalar.dma_start(wv_sb[:], wv.rearrange("p (c d) -> p c d", c=DC))
            load_x(xv_sb, xv, 0)
            load_x(xq_sb, xq, 2)
            load_x(xk_sb, xk, 2)
            load_x(xv_sb, xv, 1)
            load_x(xq_sb, xq, 3)
            load_x(xk_sb, xk, 3)
            load_x(xv_sb, xv, 2)
            load_x(xv_sb, xv, 3)
            wo_sb = pers.tile([128, DC, DIN], BF16, name="wo_sb")
            nc.gpsimd.dma_start(wo_sb[:], wo.rearrange("p (c d) -> p c d", c=DC))

            # ---- v ones columns (rowsum trick) ----
            v_sb = pers.tile([128, SKC, HL, VW], BF16)
            nc.vector.memset(v_sb[:, :, :, DK:VW], 1.0)

            # ---- projections ----
            qt_sb = [pers.tile([128, S], BF16, name=f"qt{p}") for p in range(2)]
            kt_sb = [pers.tile([128, S], BF16, name=f"kt{p}") for p in range(2)]

            def emit_qk(p):
                for sb in range(4):
                    for xsb, wsb, bsb, dst in (
                        (xq_sb, wq_sb, bq_sb, qt_sb),
                        (xk_sb, wk_sb, bk_sb, kt_sb),
                    ):
                        ps = pspj.tile([128, 512], F32, tag="pj", name="psqk")
                        for c in range(DC):
                            nc.tensor.matmul(
                                ps[:],
                                wsb[:, c, 128 * p : 128 * (p + 1)],
                                xsb[:, sb, c, :],
                                start=(c == 0),
                                stop=(c == DC - 1),
                            )
                        nc.vector.tensor_scalar_add(
                            dst[p][:, 512 * sb : 512 * (sb + 1)], ps[:], bsb[:, p : p + 1]
                        )

            def emit_v(scs):
                for sc in scs:
                    psv = pspj.tile([128, HL, DK], F32, tag="pj", name="psv")
                    for c in range(DC):
                        nc.tensor.matmul(
                            psv[:],
                            xv_sb[:, sc // 4, c, 128 * (sc % 4) : 128 * (sc % 4 + 1)],
                            wv_sb[:, c, :],
                            start=(c == 0),
                            stop=(c == DC - 1),
                        )
                    nc.vector.tensor_add(
                        v_sb[:, sc, :, 0:DK], psv[:], bv_sb[:]
                    )

            # ---- attention for one head-pair ----
            def emit_attention(p):
                for sqb in range(4):
                    qsl = slice(512 * sqb, 512 * (sqb + 1))
                    po = [
                        psacc.tile([128, 512], F32, tag="acc", name=f"po{ch}")
                        for ch in range(2)
                    ]
                    # software-pipelined: scores(k)/exp(k) emitted one step
                    # ahead of attnv(k-1) so the in-order PE never idles at
                    # the queue head waiting for exp
                    ets = [None, None]

                    def attnv(skc):
                        for ch in range(2):
                            nc.tensor.matmul(
                                po[ch][:],
                                v_sb[:, skc, 2 * p + ch, :],
                                ets[skc % 2][:, 512 * ch : 512 * (ch + 1)],
                                start=(skc == 0),
                                stop=(skc == SKC - 1),
                            )

                    for skc in range(SKC):
                        ps2 = psmm.tile([128, 1024], F32, tag="mm", name="ps2")
                        for ch in range(2):
                            cs = slice(64 * ch, 64 * (ch + 1))
                            nc.tensor.matmul(
                                ps2[:, 512 * ch : 512 * (ch + 1)],
                                kt_sb[p][cs, 128 * skc : 128 * (skc + 1)],
                                qt_sb[p][cs, qsl],
                                start=True,
                                stop=True,
                            )
                        et = work.tile([128, 1024], BF16, tag="et", name="et")
                        ets[skc % 2] = et
                        nc.scalar.activation(
                            et[:],
                            ps2[:],
                            mybir.ActivationFunctionType.Exp,
                            bias=0.0,
                            scale=float(1.0 / np.sqrt(DK)),
                        )
                        if skc >= 1:
                            attnv(skc - 1)
                    attnv(SKC - 1)
                    for ch in range(2):
                        rcp = wrk2.tile([128, 512], F32, tag="rcp", name="rcp")
                        rlo = wrk2.tile([64, 512], F32, tag="rlo", name="rlo")
                        ot = wrk2.tile([64, 512], BF16, tag="ot", name="ot")
                        nc.vector.reciprocal_approx_fast(out=rcp[:], in_=po[ch][:])
                        nc.sync.dma_start(rlo[:], rcp[64:128, :])
                        nc.vector.tensor_mul(ot[:], po[ch][0:DK, :], rlo[:])
                        # duplicate-quarter send: dests of both batches get it,
                        # the receiver's batch-select keeps the right half
                        q_eng = nc.sync
                        for shard in (sqb, sqb + 4):
                            base = shard * 2 * DK + ch * DK
                            q_eng.dma_start(cc_in[p][base : base + DK, :], ot[:])

            def emit_a2a(p):
                nc.gpsimd.collective_compute(
                    "AllToAll",
                    mybir.AluOpType.bypass,
                    replica_groups=[[0, 1, 2, 3, 4, 5, 6, 7]],
                    ins=[cc_in[p][:, :]],
                    outs=[cc_out[p][:, :]],
                )

            # receive + batch-select: ol_sel[:, 4p+r, :] =
            #   s0*cc_out[p][chunk r] + s1*cc_out[p][chunk 4+r]
            ol_sel = pers.tile([128, DC, 512], BF16, name="ol_sel")

            def emit_recv(p):
                olr = recv.tile([128, 8, 512], BF16, tag="olr", name="olr")
                tmp = recv.tile([128, 4, 512], BF16, tag="olt", name="olt")
                # two parallel half-loads on separate HWDGE queues
                nc.sync.dma_start(
                    olr[:, 0:4, :],
                    cc_out[p][0:512, :].rearrange("(c q) s -> q c s", q=128),
                )
                nc.scalar.dma_start(
                    olr[:, 4:8, :],
                    cc_out[p][512:1024, :].rearrange("(c q) s -> q c s", q=128),
                )
                nc.vector.tensor_scalar_mul(tmp[:], olr[:, 4:8, :], sel_sb[:, 1:2])
                nc.vector.scalar_tensor_tensor(
                    ol_sel[:, 4 * p : 4 * p + 4, :],
                    olr[:, 0:4, :],
                    sel_sb[:, 0:1],
                    tmp[:],
                    mybir.AluOpType.mult,
                    mybir.AluOpType.add,
                )

            # ---- emission (priority) order ----
            emit_qk(0)
            emit_v(range(SKC))
            emit_attention(0)
            emit_a2a(0)
            emit_recv(0)
            emit_qk(1)
            emit_attention(1)
            emit_a2a(1)
            emit_recv(1)

            if dbg:
                nc.sync.dma_start(d_qt[:], qt_sb[0][:])
                nc.sync.dma_start(
                    d_v.rearrange("p (c h w) -> p c h w", c=SKC, h=HL), v_sb[:]
                )
                nc.sync.dma_start(
                    d_ols.rearrange("p (c s) -> p c s", c=DC), ol_sel[:]
                )

            # ---- output projection: out[sq, :] = sum_c ol_sel^T wo + bo ----
            for sb2 in range(SQ // 128):
                os_sb = wrk2.tile([128, DIN], F32, tag="os", name="os")
                for do in range(2):
                    g = 2 * sb2 + do
                    pool = psmm if g % 3 < 2 else pspj
                    pso = pool.tile(
                        [128, 512], F32, tag="mm" if g % 3 < 2 else "pj", name="pso"
                    )
                    for c in range(DC):
                        nc.tensor.matmul(
                            pso[:],
                            ol_sel[:, c, 128 * sb2 : 128 * (sb2 + 1)],
                            wo_sb[:, c, 512 * do : 512 * (do + 1)],
                            start=(c == 0),
                            stop=(c == DC - 1),
                        )
                    nc.vector.tensor_add(
                        os_sb[:, 512 * do : 512 * (do + 1)],
                        pso[:],
                        bo_sb[:, 512 * do : 512 * (do + 1)],
                    )
                nc.sync.dma_start(out[128 * sb2 : 128 * (sb2 + 1), :], os_sb[:])

    nc.compile()
    return nc


_NC = None


def _get_nc():
    global _NC
    if _NC is None:
        _NC = build()
    return _NC


def _pack_x(Xb):
    """[2048, 1024] f32 -> [128, 4*8*512] bf16, s-block-major SBUF layout."""
    xt = np.ascontiguousarray(Xb.T)  # [1024, 2048]
    # [c, p, sblk, s] -> [p, sblk, c, s]
    x4 = xt.reshape(DC, 128, 4, 512).transpose(1, 2, 0, 3)
    return np.ascontiguousarray(x4.reshape(128, 4 * DC * 512)).astype(BF)


def _pack_w(W4):
    """[4, 1024, 64] -> [128, 8*256] bf16 ([part, c, pair-major cols])."""
    w = W4.transpose(1, 0, 2).reshape(DIN, HL * DK)  # col = 64*h_local + d
    w = w.reshape(DC, 128, HL * DK).transpose(1, 0, 2)
    return np.ascontiguousarray(w.reshape(128, DC * HL * DK)).astype(BF)


def _pack_wo(Wo):
    """[1024, 1024] -> [128, 8*1024] bf16: chunk c'=4p+r holds rows of head
    4r+2p+hh (hh=row//64), matching ol_sel chunk layout."""
    w5 = Wo.reshape(4, 2, 2, DK, DIN)  # [r, p, hh, d, out]
    w5 = w5.transpose(2, 3, 1, 0, 4)  # [hh, d, p, r, out]
    return np.ascontiguousarray(w5.reshape(128, DC * DIN)).astype(BF)


def make_in_maps(Q, K, V, Wq, bq, Wk, bk, Wv, bv, Wo, bo):
    Q, K, V = (np.asarray(a, np.float32) for a in (Q, K, V))
    Wq, bq, Wk, bk, Wv, bv = (
        np.asarray(a, np.float32) for a in (Wq, bq, Wk, bk, Wv, bv)
    )
    Wo = np.asarray(Wo, np.float32)
    bo = np.asarray(bo, np.float32)
    xpk = [(_pack_x(Q[b]), _pack_x(K[b]), _pack_x(V[b])) for b in range(B)]
    wo_p = _pack_wo(Wo)
    bo_p = np.ascontiguousarray(np.broadcast_to(bo, (128, DIN)))
    in_maps = []
    for c in range(NCORES):
        b, g = divmod(c, 4)
        hs = slice(HL * g, HL * (g + 1))
        selv = np.zeros((128, 2), np.float32)
        selv[:, b] = 1.0
        in_maps.append(
            {
                "xq": xpk[b][0],
                "xk": xpk[b][1],
                "xv": xpk[b][2],
                "wq": _pack_w(Wq[hs]),
                "wk": _pack_w(Wk[hs]),
                "wv": _pack_w(Wv[hs]),
                "wo": wo_p,
                "bqp": np.ascontiguousarray(bq[hs].reshape(2, 128).T),
                "bkp": np.ascontiguousarray(bk[hs].reshape(2, 128).T),
                "bvr": np.ascontiguousarray(
                    np.broadcast_to(bv[hs].reshape(-1), (128, HL * DK))
                ),
                "bor": bo_p,
                "sel": selv,
            }
        )
    return in_maps


def run(nc, in_maps, **kwargs):
    return bass_utils.run_bass_kernel_spmd(
        nc, in_maps, core_ids=list(range(NCORES)), **kwargs
    )


def kernel(Q, K, V, Wq, bq, Wk, bk, Wv, bv, Wo, bo):
    nc = _get_nc()
    in_maps = make_in_maps(Q, K, V, Wq, bq, Wk, bk, Wv, bv, Wo, bo)
    res = run(nc, in_maps)
    full = np.empty((B, S, DIN), np.float32)
    for c in range(NCORES):
        b, g = divmod(c, 4)
        full[b, SQ * g : SQ * (g + 1), :] = res.results[c]["out"]
    return full


# revision 13
# speedup vs baseline: 1.0321x; 1.0321x over previous
"""Multi-head attention Trainium2 kernel (8 NeuronCores), v1.

Sharding: core c handles batch b=c//4 and head group g=c%4 (4 of 16 heads,
as 2 head-pairs p=0,1).  Fully "transposed" formulation (no on-device
transposes):
  qT/kT [dq, s] via lhsT=W-pair, rhs=X^T;  v [s, dk] via lhsT=X^T-chunk,
  rhs=Wv;  scoresT [s_k, s_q] via lhsT=kT-chunk, rhs=qT (softmax axis =
  partition dim); exp fused on ScalarE (scale=1/8); rowsum via 64 ones
  columns packed next to v (free in PE column-passes); oT [dk, s_q] is the
  lhsT the output projection wants.

Differences vs v0 baseline (359 us):
  - Host pre-arranges ALL inputs in SBUF layout and bf16 where the PE needs
    bf16: no conversion DMAs, no scatter descriptors, half the input bytes.
    X tensors are s-block-major so each 512-s-block is one contiguous DMA.
  - AllToAll (8 ranks, 2x, one per head-pair) keeps the duplicate-quarters
    send, but the receiver now selects its batch's half with 2 cheap DVE ops
    (host-supplied 0/1 scalars) instead of a zero-padded Wo: output
    projection contraction halves to 8 chunks, wo DMA halves.
  - cc_out tensors are addr_space="Shared" (fast HBM-HBM collective path).
  - Emission order = scheduler priority tuned so the kernel starts the exp
    stream ~12us in and the PE never waits on collectives until the tail.
"""

import sys

if "/opt/trn_rl_repo" not in sys.path:
    sys.path.insert(0, "/opt/trn_rl_repo")

import numpy as np
import ml_dtypes

import concourse.bass as bass  # noqa: F401
import concourse.bacc as bacc
import concourse.bass_utils as bass_utils
import concourse.mybir as mybir
import concourse.tile as tile

B, S, DIN = 2, 2048, 1024
H, DK = 16, 64
NCORES = 8
HL = 4  # heads per core
SQ = S // 4  # output rows per core
DC = DIN // 128  # 8 din chunks
SKC = S // 128  # 16 s_k chunks
VW = 2 * DK  # 128: 64 v columns + 64 ones columns (rowsum via PE)

F32 = mybir.dt.float32
BF16 = mybir.dt.bfloat16
BF = ml_dtypes.bfloat16


def build(dbg=False):
    nc = bacc.Bacc("TRN2", target_bir_lowering=False, debug=False, num_devices=NCORES)

    # ---- DRAM tensors (all host-prearranged, see make_in_maps) ----
    xq = nc.dram_tensor("xq", [128, 4 * DC * 512], BF16, kind="ExternalInput")
    xk = nc.dram_tensor("xk", [128, 4 * DC * 512], BF16, kind="ExternalInput")
    xv = nc.dram_tensor("xv", [128, 4 * DC * 512], BF16, kind="ExternalInput")
    wq = nc.dram_tensor("wq", [128, DC * 256], BF16, kind="ExternalInput")
    wk = nc.dram_tensor("wk", [128, DC * 256], BF16, kind="ExternalInput")
    wv = nc.dram_tensor("wv", [128, DC * 256], BF16, kind="ExternalInput")
    wo = nc.dram_tensor("wo", [128, DC * DIN], BF16, kind="ExternalInput")
    bqp = nc.dram_tensor("bqp", [128, 2], F32, kind="ExternalInput")
    bkp = nc.dram_tensor("bkp", [128, 2], F32, kind="ExternalInput")
    bvr = nc.dram_tensor("bvr", [128, HL * DK], F32, kind="ExternalInput")
    bor = nc.dram_tensor("bor", [128, DIN], F32, kind="ExternalInput")
    sel = nc.dram_tensor("sel", [128, 2], F32, kind="ExternalInput")
    out = nc.dram_tensor("out", [SQ, DIN], F32, kind="ExternalOutput")

    cc_in = [
        nc.dram_tensor(f"cc_in{p}", [8 * 2 * DK, SQ], BF16, kind="Internal")
        for p in range(2)
    ]
    cc_out = [
        nc.dram_tensor(f"cc_out{p}", [8 * 2 * DK, SQ], BF16, kind="Internal")
        for p in range(2)
    ]
    if dbg:
        d_qt = nc.dram_tensor("d_qt", [128, S], BF16, kind="ExternalOutput")
        d_v = nc.dram_tensor("d_v", [128, SKC * HL * VW], BF16, kind="ExternalOutput")
        d_ols = nc.dram_tensor("d_ols", [128, DC * SQ], BF16, kind="ExternalOutput")

    with tile.TileContext(nc) as tc:
        with (
            tc.tile_pool(name="pers", bufs=1) as pers,
            tc.tile_pool(name="work", bufs=3) as work,
            tc.tile_pool(name="wrk2", bufs=2) as wrk2,
            tc.tile_pool(name="recv", bufs=1) as recv,
            tc.tile_pool(name="psmm", bufs=2, space="PSUM") as psmm,
            tc.tile_pool(name="psacc", bufs=3, space="PSUM") as psacc,
            tc.tile_pool(name="pspj", bufs=1, space="PSUM") as pspj,
        ):
            # ---- small per-partition constants (sync queue) ----
            bq_sb = pers.tile([128, 2], F32)
            bk_sb = pers.tile([128, 2], F32)
            bv_sb = pers.tile([128, HL, DK], F32)
            bo_sb = pers.tile([128, DIN], F32)
            sel_sb = pers.tile([128, 2], F32)
            # ---- weights first (vector HWDGE queue, starts immediately) ----
            wq_sb = pers.tile([128, DC, 256], BF16)
            wk_sb = pers.tile([128, DC, 256], BF16)
            wv_sb = pers.tile([128, DC, 256], BF16)
            nc.scalar.dma_start(wq_sb[:], wq.rearrange("p (c d) -> p c d", c=DC))
            nc.scalar.dma_start(wk_sb[:], wk.rearrange("p (c d) -> p c d", c=DC))
            nc.sync.dma_start(bq_sb[:], bqp[:])
            nc.sync.dma_start(bk_sb[:], bkp[:])
            nc.sync.dma_start(bv_sb[:], bvr.rearrange("p (h d) -> p h d", h=HL))
            nc.sync.dma_start(bo_sb[:], bor[:])
            nc.sync.dma_start(sel_sb[:], sel[:])

            # ---- X loads, s-block-major streaming ----
            xq_sb = pers.tile([128, 4, DC, 512], BF16, name="xq_sb")
            xk_sb = pers.tile([128, 4, DC, 512], BF16, name="xk_sb")
            xv_sb = pers.tile([128, 4, DC, 512], BF16, name="xv_sb")

            def load_x(xsb, xdram, sblk):
                nc.gpsimd.dma_start(
                    xsb[:, sblk, :, :],
                    xdram[:, 4096 * sblk : 4096 * (sblk + 1)].rearrange(
                        "p (c s) -> p c s", c=DC
                    ),
                )

            load_x(xq_sb, xq, 0)
            load_x(xk_sb, xk, 0)
            load_x(xq_sb, xq, 1)
            load_x(xk_sb, xk, 1)
            nc.scalar.dma_start(wv_sb[:], wv.rearrange("p (c d) -> p c d", c=DC))
            load_x(xv_sb, xv, 0)
            load_x(xq_sb, xq, 2)
            load_x(xk_sb, xk, 2)
            load_x(xv_sb, xv, 1)
            load_x(xq_sb, xq, 3)
            load_x(xk_sb, xk, 3)
            load_x(xv_sb, xv, 2)
            load_x(xv_sb, xv, 3)
            wo_sb = pers.tile([128, DC, DIN], BF16, name="wo_sb")
            nc.gpsimd.dma_start(wo_sb[:], wo.rearrange("p (c d) -> p c d", c=DC))

            # ---- v ones columns (rowsum trick) ----
            v_sb = pers.tile([128, SKC, HL, VW], BF16)
            nc.vector.memset(v_sb[:, :, :, DK:VW], 1.0)

            # ---- projections ----
            qt_sb = [pers.tile([128, S], BF16, name=f"qt{p}") for p in range(2)]
            kt_sb = [pers.tile([128, S], BF16, name=f"kt{p}") for p in range(2)]

            def emit_qk(p):
                for sb in range(4):
                    for xsb, wsb, bsb, dst in (
                        (xq_sb, wq_sb, bq_sb, qt_sb),
                        (xk_sb, wk_sb, bk_sb, kt_sb),
                    ):
                        ps = pspj.tile([128, 512], F32, tag="pj", name="psqk")
                        for c in range(DC):
                            nc.tensor.matmul(
                                ps[:],
                                wsb[:, c, 128 * p : 128 * (p + 1)],
                                xsb[:, sb, c, :],
                                start=(c == 0),
                                stop=(c == DC - 1),
                            )
                        nc.vector.tensor_scalar_add(
                            dst[p][:, 512 * sb : 512 * (sb + 1)], ps[:], bsb[:, p : p + 1]
                        )

            def emit_v(scs):
                for sc in scs:
                    psv = pspj.tile([128, HL, DK], F32, tag="pj", name="psv")
                    for c in range(DC):
                        nc.tensor.matmul(
                            psv[:],
                            xv_sb[:, sc // 4, c, 128 * (sc % 4) : 128 * (sc % 4 + 1)],
                            wv_sb[:, c, :],
                            start=(c == 0),
                            stop=(c == DC - 1),
                        )
                    nc.vector.tensor_add(
                        v_sb[:, sc, :, 0:DK], psv[:], bv_sb[:]
                    )

            # ---- attention for one head-pair ----
            def emit_attention(p):
                for sqb in range(4):
                    qsl = slice(512 * sqb, 512 * (sqb + 1))
                    po = [
                        psacc.tile([128, 512], F32, tag="acc", name=f"po{ch}")
                        for ch in range(2)
                    ]
                    # software-pipelined: scores(k)/exp(k) emitted one step
                    # ahead of attnv(k-1) so the in-order PE never idles at
                    # the queue head waiting for exp
                    ets = [None, None]

                    def attnv(skc):
                        for ch in range(2):
                            nc.tensor.matmul(
                                po[ch][:],
                                v_sb[:, skc, 2 * p + ch, :],
                                ets[skc % 2][:, 512 * ch : 512 * (ch + 1)],
                                start=(skc == 0),
                                stop=(skc == SKC - 1),
                            )

                    for skc in range(SKC):
                        ps2 = psmm.tile([128, 1024], F32, tag="mm", name="ps2")
                        for ch in range(2):
                            cs = slice(64 * ch, 64 * (ch + 1))
                            nc.tensor.matmul(
                                ps2[:, 512 * ch : 512 * (ch + 1)],
                                kt_sb[p][cs, 128 * skc : 128 * (skc + 1)],
                                qt_sb[p][cs, qsl],
                                start=True,
                                stop=True,
                            )
                        et = work.tile([128, 1024], BF16, tag="et", name="et")
                        ets[skc % 2] = et
                        nc.scalar.activation(
                            et[:],
                            ps2[:],
                            mybir.ActivationFunctionType.Exp,
                            bias=0.0,
                            scale=float(1.0 / np.sqrt(DK)),
                        )
                        if skc >= 1:
                            attnv(skc - 1)
                    attnv(SKC - 1)
                    for ch in range(2):
                        rcp = wrk2.tile([128, 512], F32, tag="rcp", name="rcp")
                        rlo = wrk2.tile([64, 512], F32, tag="rlo", name="rlo")
                        ot = wrk2.tile([64, 512], BF16, tag="ot", name="ot")
                        nc.vector.reciprocal_approx_fast(out=rcp[:], in_=po[ch][:])
                        nc.sync.dma_start(rlo[:], rcp[64:128, :])
                        nc.vector.tensor_mul(ot[:], po[ch][0:DK, :], rlo[:])
                        # duplicate-quarter send: dests of both batches get it,
                        # the receiver's batch-select keeps the right half
                        q_eng = nc.sync
                        for shard in (sqb, sqb + 4):
                            base = shard * 2 * DK + ch * DK
                            q_eng.dma_start(cc_in[p][base : base + DK, :], ot[:])

            def emit_a2a(p):
                nc.gpsimd.collective_compute(
                    "AllToAll",
                    mybir.AluOpType.bypass,
                    replica_groups=[[0, 1, 2, 3, 4, 5, 6, 7]],
                    ins=[cc_in[p][:, :]],
                    outs=[cc_out[p][:, :]],
                )

            # receive + batch-select: ol_sel[:, 4p+r, :] =
            #   s0*cc_out[p][chunk r] + s1*cc_out[p][chunk 4+r]
            ol_sel = pers.tile([128, DC, 512], BF16, name="ol_sel")

            def emit_recv(p):
                olr = recv.tile([128, 8, 512], BF16, tag="olr", name="olr")
                tmp = recv.tile([128, 4, 512], BF16, tag="olt", name="olt")
                # half-loads only on queues that are idle at this point —
                # a recv DMA on a busy queue head-of-line-blocks everything
                # behind it until the collective lands
                nc.gpsimd.dma_start(
                    olr[:, 0:4, :],
                    cc_out[p][0:512, :].rearrange("(c q) s -> q c s", q=128),
                )
                hi_eng = nc.gpsimd if p == 0 else nc.scalar
                hi_eng.dma_start(
                    olr[:, 4:8, :],
                    cc_out[p][512:1024, :].rearrange("(c q) s -> q c s", q=128),
                )
                nc.vector.tensor_scalar_mul(tmp[:], olr[:, 4:8, :], sel_sb[:, 1:2])
                nc.vector.scalar_tensor_tensor(
                    ol_sel[:, 4 * p : 4 * p + 4, :],
                    olr[:, 0:4, :],
                    sel_sb[:, 0:1],
                    tmp[:],
                    mybir.AluOpType.mult,
                    mybir.AluOpType.add,
                )

            # ---- emission (priority) order ----
            emit_qk(0)
            emit_v(range(SKC))
            emit_attention(0)
            emit_a2a(0)
            emit_recv(0)
            emit_qk(1)
            emit_attention(1)
            emit_a2a(1)
            emit_recv(1)

            if dbg:
                nc.sync.dma_start(d_qt[:], qt_sb[0][:])
                nc.sync.dma_start(
                    d_v.rearrange("p (c h w) -> p c h w", c=SKC, h=HL), v_sb[:]
                )
                nc.sync.dma_start(
                    d_ols.rearrange("p (c s) -> p c s", c=DC), ol_sel[:]
                )

            # ---- output projection: out[sq, :] = sum_c ol_sel^T wo + bo ----
            for sb2 in range(SQ // 128):
                os_sb = wrk2.tile([128, DIN], F32, tag="os", name="os")
                for do in range(2):
                    g = 2 * sb2 + do
                    pool = psmm if g % 3 < 2 else pspj
                    pso = pool.tile(
                        [128, 512], F32, tag="mm" if g % 3 < 2 else "pj", name="pso"
                    )
                    for c in range(DC):
                        nc.tensor.matmul(
                            pso[:],
                            ol_sel[:, c, 128 * sb2 : 128 * (sb2 + 1)],
                            wo_sb[:, c, 512 * do : 512 * (do + 1)],
                            start=(c == 0),
                            stop=(c == DC - 1),
                        )
                    nc.vector.tensor_add(
                        os_sb[:, 512 * do : 512 * (do + 1)],
                        pso[:],
                        bo_sb[:, 512 * do : 512 * (do + 1)],
                    )
                nc.sync.dma_start(out[128 * sb2 : 128 * (sb2 + 1), :], os_sb[:])

    nc.compile()
    return nc


_NC = None


def _get_nc():
    global _NC
    if _NC is None:
        _NC = build()
    return _NC


def _pack_x(Xb):
    """[2048, 1024] f32 -> [128, 4*8*512] bf16, s-block-major SBUF layout."""
    xt = np.ascontiguousarray(Xb.T)  # [1024, 2048]
    # [c, p, sblk, s] -> [p, sblk, c, s]
    x4 = xt.reshape(DC, 128, 4, 512).transpose(1, 2, 0, 3)
    return np.ascontiguousarray(x4.reshape(128, 4 * DC * 512)).astype(BF)


def _pack_w(W4):
    """[4, 1024, 64] -> [128, 8*256] bf16 ([part, c, pair-major cols])."""
    w = W4.transpose(1, 0, 2).reshape(DIN, HL * DK)  # col = 64*h_local + d
    w = w.reshape(DC, 128, HL * DK).transpose(1, 0, 2)
    return np.ascontiguousarray(w.reshape(128, DC * HL * DK)).astype(BF)


def _pack_wo(Wo):
    """[1024, 1024] -> [128, 8*1024] bf16: chunk c'=4p+r holds rows of head
    4r+2p+hh (hh=row//64), matching ol_sel chunk layout."""
    w5 = Wo.reshape(4, 2, 2, DK, DIN)  # [r, p, hh, d, out]
    w5 = w5.transpose(2, 3, 1, 0, 4)  # [hh, d, p, r, out]
    return np.ascontiguousarray(w5.reshape(128, DC * DIN)).astype(BF)


def make_in_maps(Q, K, V, Wq, bq, Wk, bk, Wv, bv, Wo, bo):
    Q, K, V = (np.asarray(a, np.float32) for a in (Q, K, V))
    Wq, bq, Wk, bk, Wv, bv = (
        np.asarray(a, np.float32) for a in (Wq, bq, Wk, bk, Wv, bv)
    )
    Wo = np.asarray(Wo, np.float32)
    bo = np.asarray(bo, np.float32)
    xpk = [(_pack_x(Q[b]), _pack_x(K[b]), _pack_x(V[b])) for b in range(B)]
    wo_p = _pack_wo(Wo)
    bo_p = np.ascontiguousarray(np.broadcast_to(bo, (128, DIN)))
    in_maps = []
    for c in range(NCORES):
        b, g = divmod(c, 4)
        hs = slice(HL * g, HL * (g + 1))
        selv = np.zeros((128, 2), np.float32)
        selv[:, b] = 1.0
        in_maps.append(
            {
                "xq": xpk[b][0],
                "xk": xpk[b][1],
                "xv": xpk[b][2],
                "wq": _pack_w(Wq[hs]),
                "wk": _pack_w(Wk[hs]),
                "wv": _pack_w(Wv[hs]),
                "wo": wo_p,
                "bqp": np.ascontiguousarray(bq[hs].reshape(2, 128).T),
                "bkp": np.ascontiguousarray(bk[hs].reshape(2, 128).T),
                "bvr": np.ascontiguousarray(
                    np.broadcast_to(bv[hs].reshape(-1), (128, HL * DK))
                ),
                "bor": bo_p,
                "sel": selv,
            }
        )
    return in_maps


def run(nc, in_maps, **kwargs):
    return bass_utils.run_bass_kernel_spmd(
        nc, in_maps, core_ids=list(range(NCORES)), **kwargs
    )


def kernel(Q, K, V, Wq, bq, Wk, bk, Wv, bv, Wo, bo):
    nc = _get_nc()
    in_maps = make_in_maps(Q, K, V, Wq, bq, Wk, bk, Wv, bv, Wo, bo)
    res = run(nc, in_maps)
    full = np.empty((B, S, DIN), np.float32)
    for c in range(NCORES):
        b, g = divmod(c, 4)
        full[b, SQ * g : SQ * (g + 1), :] = res.results[c]["out"]
    return full


# revision 14
# speedup vs baseline: 1.1082x; 1.0738x over previous
"""Multi-head attention Trainium2 kernel (8 NeuronCores), v1.

Sharding: core c handles batch b=c//4 and head group g=c%4 (4 of 16 heads,
as 2 head-pairs p=0,1).  Fully "transposed" formulation (no on-device
transposes):
  qT/kT [dq, s] via lhsT=W-pair, rhs=X^T;  v [s, dk] via lhsT=X^T-chunk,
  rhs=Wv;  scoresT [s_k, s_q] via lhsT=kT-chunk, rhs=qT (softmax axis =
  partition dim); exp fused on ScalarE (scale=1/8); rowsum via 64 ones
  columns packed next to v (free in PE column-passes); oT [dk, s_q] is the
  lhsT the output projection wants.

Differences vs v0 baseline (359 us):
  - Host pre-arranges ALL inputs in SBUF layout and bf16 where the PE needs
    bf16: no conversion DMAs, no scatter descriptors, half the input bytes.
    X tensors are s-block-major so each 512-s-block is one contiguous DMA.
  - AllToAll (8 ranks, 2x, one per head-pair) keeps the duplicate-quarters
    send, but the receiver now selects its batch's half with 2 cheap DVE ops
    (host-supplied 0/1 scalars) instead of a zero-padded Wo: output
    projection contraction halves to 8 chunks, wo DMA halves.
  - cc_out tensors are addr_space="Shared" (fast HBM-HBM collective path).
  - Emission order = scheduler priority tuned so the kernel starts the exp
    stream ~12us in and the PE never waits on collectives until the tail.
"""

import sys

if "/opt/trn_rl_repo" not in sys.path:
    sys.path.insert(0, "/opt/trn_rl_repo")

import numpy as np
import ml_dtypes

import concourse.bass as bass  # noqa: F401
import concourse.bacc as bacc
import concourse.bass_utils as bass_utils
import concourse.mybir as mybir
import concourse.tile as tile

B, S, DIN = 2, 2048, 1024
H, DK = 16, 64
NCORES = 8
HL = 4  # heads per core
SQ = S // 4  # output rows per core
DC = DIN // 128  # 8 din chunks
SKC = S // 128  # 16 s_k chunks
VW = 2 * DK  # 128: 64 v columns + 64 ones columns (rowsum via PE)

F32 = mybir.dt.float32
BF16 = mybir.dt.bfloat16
BF = ml_dtypes.bfloat16


def build(dbg=False):
    nc = bacc.Bacc("TRN2", target_bir_lowering=False, debug=False, num_devices=NCORES)

    # ---- DRAM tensors (all host-prearranged, see make_in_maps) ----
    xq = nc.dram_tensor("xq", [128, 4 * DC * 512], BF16, kind="ExternalInput")
    xk = nc.dram_tensor("xk", [128, 4 * DC * 512], BF16, kind="ExternalInput")
    xv = nc.dram_tensor("xv", [128, 4 * DC * 512], BF16, kind="ExternalInput")
    wq = nc.dram_tensor("wq", [128, DC * 256], BF16, kind="ExternalInput")
    wk = nc.dram_tensor("wk", [128, DC * 256], BF16, kind="ExternalInput")
    wv = nc.dram_tensor("wv", [128, DC * 256], BF16, kind="ExternalInput")
    wo = nc.dram_tensor("wo", [128, DC * DIN], BF16, kind="ExternalInput")
    bqp = nc.dram_tensor("bqp", [128, 2], F32, kind="ExternalInput")
    bkp = nc.dram_tensor("bkp", [128, 2], F32, kind="ExternalInput")
    bvr = nc.dram_tensor("bvr", [128, HL * DK], F32, kind="ExternalInput")
    bor = nc.dram_tensor("bor", [128, DIN], F32, kind="ExternalInput")
    sel = nc.dram_tensor("sel", [128, 2], F32, kind="ExternalInput")
    out = nc.dram_tensor("out", [SQ, DIN], F32, kind="ExternalOutput")

    cc_in = [
        nc.dram_tensor(f"cc_in{p}", [8 * 2 * DK, SQ], BF16, kind="Internal")
        for p in range(2)
    ]
    cc_out = [
        nc.dram_tensor(f"cc_out{p}", [8 * 2 * DK, SQ], BF16, kind="Internal")
        for p in range(2)
    ]
    if dbg:
        d_qt = nc.dram_tensor("d_qt", [128, S], BF16, kind="ExternalOutput")
        d_v = nc.dram_tensor("d_v", [128, SKC * HL * VW], BF16, kind="ExternalOutput")
        d_ols = nc.dram_tensor("d_ols", [128, DC * SQ], BF16, kind="ExternalOutput")

    with tile.TileContext(nc) as tc:
        with (
            tc.tile_pool(name="pers", bufs=1) as pers,
            tc.tile_pool(name="work", bufs=3) as work,
            tc.tile_pool(name="wrk2", bufs=2) as wrk2,
            tc.tile_pool(name="recv", bufs=1) as recv,
            tc.tile_pool(name="psmm", bufs=2, space="PSUM") as psmm,
            tc.tile_pool(name="psacc", bufs=3, space="PSUM") as psacc,
            tc.tile_pool(name="pspj", bufs=1, space="PSUM") as pspj,
        ):
            # ---- small per-partition constants (sync queue) ----
            bq_sb = pers.tile([128, 2], F32)
            bk_sb = pers.tile([128, 2], F32)
            bv_sb = pers.tile([128, HL, DK], F32)
            bo_sb = pers.tile([128, DIN], F32)
            sel_sb = pers.tile([128, 2], F32)
            # ---- weights first (vector HWDGE queue, starts immediately) ----
            wq_sb = pers.tile([128, DC, 256], BF16)
            wk_sb = pers.tile([128, DC, 256], BF16)
            wv_sb = pers.tile([128, DC, 256], BF16)
            nc.scalar.dma_start(wq_sb[:], wq.rearrange("p (c d) -> p c d", c=DC))
            nc.scalar.dma_start(wk_sb[:], wk.rearrange("p (c d) -> p c d", c=DC))
            nc.sync.dma_start(bq_sb[:], bqp[:])
            nc.sync.dma_start(bk_sb[:], bkp[:])
            nc.sync.dma_start(bv_sb[:], bvr.rearrange("p (h d) -> p h d", h=HL))
            nc.sync.dma_start(bo_sb[:], bor[:])
            nc.sync.dma_start(sel_sb[:], sel[:])

            # ---- X loads, s-block-major streaming ----
            xq_sb = pers.tile([128, 4, DC, 512], BF16, name="xq_sb")
            xk_sb = pers.tile([128, 4, DC, 512], BF16, name="xk_sb")
            xv_sb = pers.tile([128, 4, DC, 512], BF16, name="xv_sb")

            def load_x(xsb, xdram, sblk):
                nc.gpsimd.dma_start(
                    xsb[:, sblk, :, :],
                    xdram[:, 4096 * sblk : 4096 * (sblk + 1)].rearrange(
                        "p (c s) -> p c s", c=DC
                    ),
                )

            load_x(xq_sb, xq, 0)
            load_x(xk_sb, xk, 0)
            load_x(xq_sb, xq, 1)
            load_x(xk_sb, xk, 1)
            nc.scalar.dma_start(wv_sb[:], wv.rearrange("p (c d) -> p c d", c=DC))
            load_x(xv_sb, xv, 0)
            load_x(xq_sb, xq, 2)
            load_x(xk_sb, xk, 2)
            load_x(xv_sb, xv, 1)
            load_x(xq_sb, xq, 3)
            load_x(xk_sb, xk, 3)
            load_x(xv_sb, xv, 2)
            load_x(xv_sb, xv, 3)
            wo_sb = pers.tile([128, DC, DIN], BF16, name="wo_sb")
            nc.gpsimd.dma_start(wo_sb[:], wo.rearrange("p (c d) -> p c d", c=DC))

            # ---- v ones columns (rowsum trick) ----
            v_sb = pers.tile([128, SKC, HL, VW], BF16)
            nc.vector.memset(v_sb[:, :, :, DK:VW], 1.0)

            # ---- projections ----
            qt_sb = [pers.tile([128, S], BF16, name=f"qt{p}") for p in range(2)]
            kt_sb = [pers.tile([128, S], BF16, name=f"kt{p}") for p in range(2)]

            def emit_qk(p):
                for sb in range(4):
                    for xsb, wsb, bsb, dst in (
                        (xq_sb, wq_sb, bq_sb, qt_sb),
                        (xk_sb, wk_sb, bk_sb, kt_sb),
                    ):
                        ps = pspj.tile([128, 512], F32, tag="pj", name="psqk")
                        for c in range(DC):
                            nc.tensor.matmul(
                                ps[:],
                                wsb[:, c, 128 * p : 128 * (p + 1)],
                                xsb[:, sb, c, :],
                                start=(c == 0),
                                stop=(c == DC - 1),
                            )
                        nc.vector.tensor_scalar_add(
                            dst[p][:, 512 * sb : 512 * (sb + 1)], ps[:], bsb[:, p : p + 1]
                        )

            def emit_v(scs):
                for sc in scs:
                    psv = pspj.tile([128, HL, DK], F32, tag="pj", name="psv")
                    for c in range(DC):
                        nc.tensor.matmul(
                            psv[:],
                            xv_sb[:, sc // 4, c, 128 * (sc % 4) : 128 * (sc % 4 + 1)],
                            wv_sb[:, c, :],
                            start=(c == 0),
                            stop=(c == DC - 1),
                        )
                    nc.vector.tensor_add(
                        v_sb[:, sc, :, 0:DK], psv[:], bv_sb[:]
                    )

            # ---- attention for one head-pair ----
            def emit_attention(p):
                for sqb in range(4):
                    qsl = slice(512 * sqb, 512 * (sqb + 1))
                    po = [
                        psacc.tile([128, 512], F32, tag="acc", name=f"po{ch}")
                        for ch in range(2)
                    ]
                    # software-pipelined: scores(k)/exp(k) emitted one step
                    # ahead of attnv(k-1) so the in-order PE never idles at
                    # the queue head waiting for exp
                    ets = [None, None]

                    def attnv(skc):
                        for ch in range(2):
                            nc.tensor.matmul(
                                po[ch][:],
                                v_sb[:, skc, 2 * p + ch, :],
                                ets[skc % 2][:, 512 * ch : 512 * (ch + 1)],
                                start=(skc == 0),
                                stop=(skc == SKC - 1),
                            )

                    for skc in range(SKC):
                        ps2 = psmm.tile([128, 1024], F32, tag="mm", name="ps2")
                        for ch in range(2):
                            cs = slice(64 * ch, 64 * (ch + 1))
                            nc.tensor.matmul(
                                ps2[:, 512 * ch : 512 * (ch + 1)],
                                kt_sb[p][cs, 128 * skc : 128 * (skc + 1)],
                                qt_sb[p][cs, qsl],
                                start=True,
                                stop=True,
                            )
                        et = work.tile([128, 1024], BF16, tag="et", name="et")
                        ets[skc % 2] = et
                        nc.scalar.activation(
                            et[:],
                            ps2[:],
                            mybir.ActivationFunctionType.Exp,
                            bias=0.0,
                            scale=float(1.0 / np.sqrt(DK)),
                        )
                        if skc >= 1:
                            attnv(skc - 1)
                    attnv(SKC - 1)
                    for ch in range(2):
                        rcp = wrk2.tile([128, 512], F32, tag="rcp", name="rcp")
                        rlo = wrk2.tile([64, 512], F32, tag="rlo", name="rlo")
                        ot = wrk2.tile([64, 512], BF16, tag="ot", name="ot")
                        nc.vector.reciprocal_approx_fast(out=rcp[:], in_=po[ch][:])
                        nc.sync.dma_start(rlo[:], rcp[64:128, :])
                        nc.vector.tensor_mul(ot[:], po[ch][0:DK, :], rlo[:])
                        # duplicate-quarter send: dests of both batches get it,
                        # the receiver's batch-select keeps the right half
                        q_eng = nc.sync
                        for shard in (sqb, sqb + 4):
                            base = shard * 2 * DK + ch * DK
                            q_eng.dma_start(cc_in[p][base : base + DK, :], ot[:])

            def emit_a2a(p):
                nc.gpsimd.collective_compute(
                    "AllToAll",
                    mybir.AluOpType.bypass,
                    replica_groups=[[0, 1, 2, 3, 4, 5, 6, 7]],
                    ins=[cc_in[p][:, :]],
                    outs=[cc_out[p][:, :]],
                )

            # receive + batch-select: ol_sel[:, 4p+r, :] =
            #   s0*cc_out[p][chunk r] + s1*cc_out[p][chunk 4+r]
            ol_sel = pers.tile([128, DC, 512], BF16, name="ol_sel")

            def emit_recv(p):
                olr = recv.tile([128, 8, 512], BF16, tag="olr", name="olr")
                tmp = recv.tile([128, 4, 512], BF16, tag="olt", name="olt")
                # half-loads only on queues that are idle at this point —
                # a recv DMA on a busy queue head-of-line-blocks everything
                # behind it until the collective lands
                nc.gpsimd.dma_start(
                    olr[:, 0:4, :],
                    cc_out[p][0:512, :].rearrange("(c q) s -> q c s", q=128),
                )
                hi_eng = nc.gpsimd if p == 0 else nc.scalar
                hi_eng.dma_start(
                    olr[:, 4:8, :],
                    cc_out[p][512:1024, :].rearrange("(c q) s -> q c s", q=128),
                )
                nc.vector.tensor_scalar_mul(tmp[:], olr[:, 4:8, :], sel_sb[:, 1:2])
                nc.vector.scalar_tensor_tensor(
                    ol_sel[:, 4 * p : 4 * p + 4, :],
                    olr[:, 0:4, :],
                    sel_sb[:, 0:1],
                    tmp[:],
                    mybir.AluOpType.mult,
                    mybir.AluOpType.add,
                )

            # ---- emission (priority) order ----
            emit_qk(0)
            emit_v(range(SKC))
            emit_attention(0)
            emit_a2a(0)
            emit_qk(1)
            emit_attention(1)
            # recv(0) emitted only now: its DVE select would otherwise
            # head-of-line-block qk(1)'s bias adds (and with them the whole
            # second attention) on the DVE queue until a2a(0) lands
            emit_recv(0)
            emit_a2a(1)
            emit_recv(1)

            if dbg:
                nc.sync.dma_start(d_qt[:], qt_sb[0][:])
                nc.sync.dma_start(
                    d_v.rearrange("p (c h w) -> p c h w", c=SKC, h=HL), v_sb[:]
                )
                nc.sync.dma_start(
                    d_ols.rearrange("p (c s) -> p c s", c=DC), ol_sel[:]
                )

            # ---- output projection: out[sq, :] = sum_c ol_sel^T wo + bo ----
            for sb2 in range(SQ // 128):
                os_sb = wrk2.tile([128, DIN], F32, tag="os", name="os")
                for do in range(2):
                    g = 2 * sb2 + do
                    pool = psmm if g % 3 < 2 else pspj
                    pso = pool.tile(
                        [128, 512], F32, tag="mm" if g % 3 < 2 else "pj", name="pso"
                    )
                    for c in range(DC):
                        nc.tensor.matmul(
                            pso[:],
                            ol_sel[:, c, 128 * sb2 : 128 * (sb2 + 1)],
                            wo_sb[:, c, 512 * do : 512 * (do + 1)],
                            start=(c == 0),
                            stop=(c == DC - 1),
                        )
                    nc.vector.tensor_add(
                        os_sb[:, 512 * do : 512 * (do + 1)],
                        pso[:],
                        bo_sb[:, 512 * do : 512 * (do + 1)],
                    )
                nc.sync.dma_start(out[128 * sb2 : 128 * (sb2 + 1), :], os_sb[:])

    nc.compile()
    return nc


_NC = None


def _get_nc():
    global _NC
    if _NC is None:
        _NC = build()
    return _NC


def _pack_x(Xb):
    """[2048, 1024] f32 -> [128, 4*8*512] bf16, s-block-major SBUF layout."""
    xt = np.ascontiguousarray(Xb.T)  # [1024, 2048]
    # [c, p, sblk, s] -> [p, sblk, c, s]
    x4 = xt.reshape(DC, 128, 4, 512).transpose(1, 2, 0, 3)
    return np.ascontiguousarray(x4.reshape(128, 4 * DC * 512)).astype(BF)


def _pack_w(W4):
    """[4, 1024, 64] -> [128, 8*256] bf16 ([part, c, pair-major cols])."""
    w = W4.transpose(1, 0, 2).reshape(DIN, HL * DK)  # col = 64*h_local + d
    w = w.reshape(DC, 128, HL * DK).transpose(1, 0, 2)
    return np.ascontiguousarray(w.reshape(128, DC * HL * DK)).astype(BF)


def _pack_wo(Wo):
    """[1024, 1024] -> [128, 8*1024] bf16: chunk c'=4p+r holds rows of head
    4r+2p+hh (hh=row//64), matching ol_sel chunk layout."""
    w5 = Wo.reshape(4, 2, 2, DK, DIN)  # [r, p, hh, d, out]
    w5 = w5.transpose(2, 3, 1, 0, 4)  # [hh, d, p, r, out]
    return np.ascontiguousarray(w5.reshape(128, DC * DIN)).astype(BF)


def make_in_maps(Q, K, V, Wq, bq, Wk, bk, Wv, bv, Wo, bo):
    Q, K, V = (np.asarray(a, np.float32) for a in (Q, K, V))
    Wq, bq, Wk, bk, Wv, bv = (
        np.asarray(a, np.float32) for a in (Wq, bq, Wk, bk, Wv, bv)
    )
    Wo = np.asarray(Wo, np.float32)
    bo = np.asarray(bo, np.float32)
    xpk = [(_pack_x(Q[b]), _pack_x(K[b]), _pack_x(V[b])) for b in range(B)]
    wo_p = _pack_wo(Wo)
    bo_p = np.ascontiguousarray(np.broadcast_to(bo, (128, DIN)))
    in_maps = []
    for c in range(NCORES):
        b, g = divmod(c, 4)
        hs = slice(HL * g, HL * (g + 1))
        selv = np.zeros((128, 2), np.float32)
        selv[:, b] = 1.0
        in_maps.append(
            {
                "xq": xpk[b][0],
                "xk": xpk[b][1],
                "xv": xpk[b][2],
                "wq": _pack_w(Wq[hs]),
                "wk": _pack_w(Wk[hs]),
                "wv": _pack_w(Wv[hs]),
                "wo": wo_p,
                "bqp": np.ascontiguousarray(bq[hs].reshape(2, 128).T),
                "bkp": np.ascontiguousarray(bk[hs].reshape(2, 128).T),
                "bvr": np.ascontiguousarray(
                    np.broadcast_to(bv[hs].reshape(-1), (128, HL * DK))
                ),
                "bor": bo_p,
                "sel": selv,
            }
        )
    return in_maps


def run(nc, in_maps, **kwargs):
    return bass_utils.run_bass_kernel_spmd(
        nc, in_maps, core_ids=list(range(NCORES)), **kwargs
    )


def kernel(Q, K, V, Wq, bq, Wk, bk, Wv, bv, Wo, bo):
    nc = _get_nc()
    in_maps = make_in_maps(Q, K, V, Wq, bq, Wk, bk, Wv, bv, Wo, bo)
    res = run(nc, in_maps)
    full = np.empty((B, S, DIN), np.float32)
    for c in range(NCORES):
        b, g = divmod(c, 4)
        full[b, SQ * g : SQ * (g + 1), :] = res.results[c]["out"]
    return full
